# revision 11
# baseline (speedup 1.0000x reference)
"""GQA attention block (B=2, T=2048, C=2048, H=32, Hkv=8, D=64, RoPE, causal)
on 8 TRN2 NeuronCores.

Sharding: core = b*4 + g  (b = batch 0..1, g = head-group 0..3).
Each core computes 8 Q heads / 2 KV heads of one batch element:
  QKV projections -> RoPE -> causal softmax(QK^T/sqrt(D)) V -> partial
  output projection against its 512 columns of Wc.  Host sums the 4
  head-group partials per batch.

v3 pipeline:
  * All DRAM inputs are host-pre-arranged so every DMA is contiguous per
    partition (hardware descriptor generation, few dispatches).
  * Scores S^T = K Q^T (k on partitions, 2 heads in PE row groups 0/64);
    exp on ScalarE; PV with V+ones stationary (M=65; the ones column
    accumulates the softmax denominator as psum row 64).
  * RoPE rotate-half is a PE matmul against a 0/1 permutation matrix
    (no partition-shuffle DMAs).
  * Emission is software-pipelined: PV lags exp by one k-tile, and the
    next block's projection chunks plus the previous block's output-
    projection chunks are interleaved into the attention loop as PE
    filler so the in-order PE queue never blocks on ScalarE exp.
  * Output is written bf16 (host sums partials in fp32).
"""

import numpy as np

import ml_dtypes

import concourse.bacc as bacc
import concourse.mybir as mybir
from concourse.tile import TileContext
from concourse.bass_utils import run_bass_kernel_spmd

B, T, C = 2, 2048, 2048
H, HKV, D = 32, 8, 64
ROPE_THETA = 10000.0

P = 128
NCT = C // P          # 16 contraction subtiles
TB = 512              # t-block width
NTB = T // TB         # 4
QB = 512              # q-block width in attention
KT = T // P           # 16 k-tiles
QH = H // 4           # 8 local q heads per core
LOCAL_HEADS = [0, 4, 1, 5, 2, 6, 3, 7]  # pair (p, p+4) shares a 128-row tile

F32 = mybir.dt.float32
BF16 = mybir.dt.bfloat16
NPDT = ml_dtypes.bfloat16

EXP_SCALE = float(1.0 / np.sqrt(D))


def build_bass():
    nc = bacc.Bacc("TRN2", target_bir_lowering=False, debug=False, num_devices=8)

    # host-pre-arranged, fully contiguous per partition
    xd = nc.dram_tensor("xd", [P, NTB, NCT, TB], BF16, kind="ExternalInput")
    wqd = nc.dram_tensor("wqd", [P, 4, NCT, P], BF16, kind="ExternalInput")
    wkd = nc.dram_tensor("wkd", [P, NCT, P], BF16, kind="ExternalInput")
    wvd = nc.dram_tensor("wvd", [P, NCT, P], BF16, kind="ExternalInput")
    wcd = nc.dram_tensor("wcd", [P, 4, C], BF16, kind="ExternalInput")
    cosd = nc.dram_tensor("cosd", [P, T], BF16, kind="ExternalInput")
    sind = nc.dram_tensor("sind", [P, T], BF16, kind="ExternalInput")
    rotd = nc.dram_tensor("rotd", [P, P], BF16, kind="ExternalInput")
    trid = nc.dram_tensor("trid", [P, P], BF16, kind="ExternalInput")
    identd = nc.dram_tensor("identd", [P, P], BF16, kind="ExternalInput")
    out = nc.dram_tensor("out", [T, C], BF16, kind="ExternalOutput")

    with TileContext(nc) as tc:
        with (
            tc.tile_pool(name="persist", bufs=1) as persist,
            tc.tile_pool(name="xc", bufs=3) as xcp,
            tc.tile_pool(name="rope", bufs=3) as rotp,
            tc.tile_pool(name="vt", bufs=2) as vtp,
            tc.tile_pool(name="pt", bufs=6) as ptp,
            tc.tile_pool(name="pvc", bufs=2) as pvcp,
            tc.tile_pool(name="small", bufs=4) as small,
            tc.tile_pool(name="ostage", bufs=2) as ostage,
            tc.tile_pool(name="psMM", bufs=2, space="PSUM") as psMM,
            tc.tile_pool(name="psST", bufs=2, space="PSUM") as psST,
            tc.tile_pool(name="psPV", bufs=1, space="PSUM") as psPV,
        ):
            # ---- persistent SBUF tensors ------------------------------
            q_sb = persist.tile([P, 4, T], BF16)          # Q^T (rope'd)
            k_sb = persist.tile([P, T], BF16)             # K^T (rope'd)
            v_sb = persist.tile([P, KT, 2, D + 1], BF16)  # V + ones col
            y_sb = persist.tile([P, 4, T], BF16)          # attn out^T
            rot_sb = persist.tile([P, P], BF16)
            tri_sb = persist.tile([P, P], BF16)
            id_sb = persist.tile([P, P], BF16)
            cos_sb = persist.tile([P, T], BF16)
            sin_sb = persist.tile([P, T], BF16)
            wk_sb = persist.tile([P, NCT, P], BF16, tag="wk")
            wv_sb = persist.tile([P, NCT, P], BF16, tag="wv")
            wq_sb = persist.tile([P, 4, NCT, P], BF16, tag="wq")
            wc_sb = persist.tile([P, 4, C], BF16, tag="wc")
            x0a = persist.tile([P, 2, TB], BF16, tag="x0a")
            x0b = persist.tile([P, 6, TB], BF16, tag="x0b")

            # x chunk resolver: block 0 is split 2/6/8 so the first
            # projection matmuls gate on a small transfer; blocks 1-3 are
            # two 8-chunk tiles each.
            x_half = {}

            def xc(tb, c):
                if tb == 0:
                    if c < 2:
                        return x0a[:, c, :]
                    if c < 8:
                        return x0b[:, c - 2, :]
                    return x_half[(0, 1)][:, c - 8, :]
                return x_half[(tb, c // 8)][:, c % 8, :]

            def load_x_half(tb, h):
                t = xcp.tile([P, 8, TB], BF16, tag="xh", name=f"xh{tb}_{h}")
                nc.sync.dma_start(t[:], xd[:, tb, h * 8 : h * 8 + 8, :])
                x_half[(tb, h)] = t

            # startup DMAs, gating order
            nc.sync.dma_start(wk_sb[:], wkd[:])
            nc.sync.dma_start(x0a[:], xd[:, 0, 0:2, :])
            nc.sync.dma_start(wq_sb[:, 0], wqd[:, 0])
            nc.sync.dma_start(x0b[:], xd[:, 0, 2:8, :])
            nc.sync.dma_start(cos_sb[:], cosd[:])
            nc.sync.dma_start(sin_sb[:], sind[:])
            nc.sync.dma_start(rot_sb[:], rotd[:])
            load_x_half(0, 1)
            nc.sync.dma_start(wq_sb[:, 1], wqd[:, 1])
            nc.sync.dma_start(wv_sb[:], wvd[:])
            nc.sync.dma_start(wq_sb[:, 2], wqd[:, 2])
            nc.sync.dma_start(wq_sb[:, 3], wqd[:, 3])
            nc.sync.dma_start(tri_sb[:], trid[:])
            nc.sync.dma_start(id_sb[:], identd[:])
            nc.gpsimd.dma_start(wc_sb[:], wcd[:])
            nc.vector.memset(v_sb[:, :, :, D], 1.0)

            def rope_store(dst, psum, tb):
                # q_rope = q*cos + rot(q)*sin'; rotate-half via PE perm MM
                tmp = rotp.tile([P, TB], BF16, tag="rp_t")
                nc.vector.tensor_copy(tmp[:], psum[:])
                rps = psMM.tile([P, TB], F32, tag="mm512", name="rot")
                nc.tensor.matmul(rps[:], rot_sb[:], tmp[:], start=True, stop=True)
                ts = slice(tb * TB, (tb + 1) * TB)
                rt = rotp.tile([P, TB], BF16, tag="rp_r")
                nc.vector.tensor_mul(rt[:], rps[:], sin_sb[:, ts])
                nc.vector.tensor_mul(dst, tmp[:], cos_sb[:, ts])
                nc.vector.tensor_add(dst, dst, rt[:])

            def proj_chunks(tb):
                """Yield filler closures for t-block tb's projections.
                m-major: each psum's 16 contraction matmuls are emitted in
                two 8-MM units, then its rope/finish unit, so at most two
                mm512 psum tiles are ever live (psMM bufs=2)."""
                tsl = slice(tb * TB, (tb + 1) * TB)
                state = {}

                def start():
                    if tb > 0:
                        load_x_half(tb, 0)
                        load_x_half(tb, 1)

                yield start

                def mk_mms(which, m, chalf):
                    def emit():
                        if chalf == 0:
                            key = f"{which}{m}"
                            state[key] = psMM.tile(
                                [P, TB], F32, tag="mm512", name=key
                            )
                        ps = state[f"{which}{m}"]
                        w_sb = {"k": wk_sb, "v": wv_sb}.get(which)
                        for c in range(chalf * 8, chalf * 8 + 8):
                            lhs = (
                                wq_sb[:, m, c, :]
                                if which == "q"
                                else w_sb[:, c, :]
                            )
                            nc.tensor.matmul(
                                ps[:], lhs, xc(tb, c),
                                start=(c == 0), stop=(c == NCT - 1),
                            )
                    return emit

                def mk_rope(which, m):
                    def emit():
                        dst = (
                            k_sb[:, tsl] if which == "k" else q_sb[:, m, tsl]
                        )
                        rope_store(dst, state[f"{which}{m}"], tb)
                    return emit

                yield mk_mms("k", 0, 0)
                yield mk_mms("k", 0, 1)
                yield mk_rope("k", 0)
                for m in range(4):
                    yield mk_mms("q", m, 0)
                    yield mk_mms("q", m, 1)
                    yield mk_rope("q", m)
                yield mk_mms("v", 0, 0)
                yield mk_mms("v", 0, 1)

                def v_fin():
                    vt_sb = vtp.tile([P, TB], BF16, tag="vt")
                    nc.vector.tensor_copy(vt_sb[:], state["v0"][:])
                    for s in range(TB // P):
                        kt = tb * (TB // P) + s
                        ptr = psMM.tile([P, P], BF16, tag="mm512", name="ptr")
                        nc.tensor.transpose(
                            ptr[:], vt_sb[:, s * P : (s + 1) * P], id_sb[:]
                        )
                        nc.vector.tensor_copy(v_sb[:, kt, :, 0:D], ptr[:])
                yield v_fin

            def outproj_chunks(jq):
                """Output projection for rows jq*TB..(jq+1)*TB, as filler
                units: one unit per (t-tile, column-block) = 4 matmuls +
                a psum->sbuf copy; plus a DMA unit per t-tile."""
                for s in range(4):
                    t = jq * 4 + s
                    ob = {}

                    def mk_alloc(ob=ob):
                        def emit():
                            ob["t"] = ostage.tile(
                                [P, C], BF16, tag="ob", name="ob"
                            )
                        return emit

                    yield mk_alloc()

                    def mk_cb(cb, t=t, ob=ob):
                        def emit():
                            csl = slice(cb * 512, (cb + 1) * 512)
                            po = psMM.tile([P, 512], F32, tag="mm512", name="po")
                            for jj in range(4):
                                nc.tensor.matmul(
                                    po[:],
                                    y_sb[:, jj, t * P : (t + 1) * P],
                                    wc_sb[:, jj, csl],
                                    start=(jj == 0),
                                    stop=(jj == 3),
                                )
                            if cb % 2 == 0:
                                nc.scalar.copy(ob["t"][:, csl], po[:])
                            else:
                                nc.vector.tensor_copy(ob["t"][:, csl], po[:])
                        return emit

                    for cb in range(4):
                        yield mk_cb(cb)

                    def mk_dma(t=t, ob=ob):
                        def emit():
                            nc.gpsimd.dma_start(
                                out[t * P : (t + 1) * P, :], ob["t"][:]
                            )
                        return emit

                    yield mk_dma()

            def attn_block(jq, fillers):
                """Causal attention for q rows [jq*QB, (jq+1)*QB).  PV lags
                exp by one k-tile; `fillers` are drained between
                iterations at an even cadence."""
                qb = jq * QB
                nkt = 4 * jq + 4
                iters = 4 * nkt
                fill_i = 0
                it = 0

                def fill():
                    # proportional drain: after iteration `it`, exactly
                    # ceil(it * len / iters) filler units have been emitted
                    nonlocal fill_i
                    target = -(-(it * len(fillers)) // iters)
                    while fill_i < min(target, len(fillers)):
                        fillers[fill_i]()
                        fill_i += 1

                for pr in range(4):  # head-pair tiles (local heads pr, pr+4)
                    pv = psPV.tile([D + 1, 2, QB], F32, tag="pv")
                    pending = None

                    def emit_pv(ptile, kt, j, w):
                        qo = P * j if j > 0 else 0
                        for hh in range(2):
                            nc.tensor.matmul(
                                pv[:, hh, qo : qo + w],
                                v_sb[:, kt, hh, :],
                                ptile[:, hh, 0:w],
                                start=(kt == 0),
                                stop=(kt == nkt - 1),
                            )

                    for kt in range(nkt):
                        j = kt - 4 * jq  # >= 0: diagonal-crossing tile
                        w = QB - P * j if j >= 0 else QB
                        qoff = qb + P * j if j >= 0 else qb
                        ksl = slice(kt * P, (kt + 1) * P)
                        st = psST.tile([P, 2, QB], F32, tag="st")
                        for hh in range(2):
                            hsl = slice(hh * D, (hh + 1) * D)
                            nc.tensor.matmul(
                                st[:, hh, 0:w],
                                k_sb[hsl, ksl],
                                q_sb[hsl, pr, qoff : qoff + w],
                                start=True,
                                stop=True,
                            )
                        ptile = ptp.tile([P, 2, QB], BF16, tag="pt")
                        nc.scalar.activation(
                            ptile[:, :, 0:w],
                            st[:, :, 0:w],
                            mybir.ActivationFunctionType.Exp,
                            scale=EXP_SCALE,
                        )
                        if j >= 0:
                            nc.vector.tensor_mul(
                                ptile[:, :, 0:P],
                                ptile[:, :, 0:P],
                                tri_sb[:, None, :].to_broadcast((P, 2, P)),
                            )
                        if pending is not None:
                            emit_pv(*pending)
                        pending = (ptile, kt, j, w)
                        it += 1
                        fill()
                    emit_pv(*pending)

                    # ---- normalize by the denominator row -------------
                    pvcb = pvcp.tile([D + 1, 2, QB], F32, tag="pvc")
                    nc.vector.tensor_copy(pvcb[:], pv[:])
                    for hh in range(2):
                        srow = small.tile([1, QB], F32, tag="srow")
                        nc.vector.tensor_copy(srow[:], pv[D : D + 1, hh, :])
                        rec = small.tile([1, QB], F32, tag="rec")
                        nc.vector.reciprocal_approx_fast(rec[:], srow[:])
                        bc = small.tile([D, QB], F32, tag="bc")
                        nc.gpsimd.partition_broadcast(bc[:], rec[:])
                        nc.vector.tensor_mul(
                            y_sb[hh * D : (hh + 1) * D, pr, qb : qb + QB],
                            pvcb[0:D, hh, :],
                            bc[:],
                        )
                while fill_i < len(fillers):
                    fillers[fill_i]()
                    fill_i += 1

            # ---- emission ------------------------------------------------
            for ch in proj_chunks(0):
                ch()
            for tb in range(NTB):
                fillers = []
                if tb + 1 < NTB:
                    fillers.extend(proj_chunks(tb + 1))
                if tb > 0:
                    fillers.extend(outproj_chunks(tb - 1))
                attn_block(tb, fillers)
            for ch in outproj_chunks(NTB - 1):
                ch()

    nc.finalize()
    return nc


def _rope_tables(position_ids):
    t = position_ids.reshape(-1).astype(np.float64)  # [T]
    inv_freq = 1.0 / ROPE_THETA ** (np.arange(0, D, 2, dtype=np.float64) / D)
    freqs = np.outer(t, inv_freq)  # [T, D/2]
    cos = np.repeat(np.cos(freqs), 2, axis=1)  # [T, D] interleaved
    sin = np.repeat(np.sin(freqs), 2, axis=1)
    sign = np.where(np.arange(D) < D // 2, -1.0, 1.0)
    cosT = np.tile(cos.T, (2, 1))            # [128, T]
    sinT = np.tile((sin * sign).T, (2, 1))   # [128, T]
    return cosT.astype(NPDT), sinT.astype(NPDT)


def _rot_perm():
    # out = perm.T @ in : out[olo+r] = in[ilo+r]
    perm = np.zeros((P, P), dtype=np.float32)
    for olo, ilo in ((0, 32), (32, 0), (64, 96), (96, 64)):
        for r in range(32):
            perm[ilo + r, olo + r] = 1.0
    return perm


def _head_perm(g):
    # row indices into Wq (and columns of Wc) for core head-group g
    rows = []
    for lh in LOCAL_HEADS:
        h = g * QH + lh
        rows.extend(range(h * D, (h + 1) * D))
    return np.asarray(rows)


def make_in_maps(x, Wq, Wk, Wv, Wc, position_ids):
    x = np.asarray(x, dtype=np.float32)
    Wq = np.asarray(Wq, dtype=np.float32)
    Wk = np.asarray(Wk, dtype=np.float32)
    Wv = np.asarray(Wv, dtype=np.float32)
    Wc = np.asarray(Wc, dtype=np.float32)
    cosT, sinT = _rope_tables(np.asarray(position_ids))
    tri = np.triu(np.ones((P, P), dtype=np.float32))  # allow q >= k
    in_maps = []
    for core in range(8):
        b, g = divmod(core, 4)
        perm = _head_perm(g)
        kv = slice(2 * g * D, (2 * g + 2) * D)
        # x: [C, T] -> [o 16, p 128, tb 4, 512] -> [p, tb, o, 512]
        xT = x[b].T.reshape(NCT, P, NTB, TB).transpose(1, 2, 0, 3)
        # wq: [C, 512] -> [o, p, m 4, 128] -> [p, m, o, 128]
        wqT = Wq[perm].T.reshape(NCT, P, 4, P).transpose(1, 2, 0, 3)
        wkT = Wk[kv].T.reshape(NCT, P, P).transpose(1, 0, 2)
        wvT = Wv[kv].T.reshape(NCT, P, P).transpose(1, 0, 2)
        # wc: [512, C] -> [jj 4, p, C] -> [p, jj, C]
        wcT = Wc[:, perm].T.reshape(4, P, C).transpose(1, 0, 2)
        in_maps.append(
            {
                "xd": np.ascontiguousarray(xT).astype(NPDT),
                "wqd": np.ascontiguousarray(wqT).astype(NPDT),
                "wkd": np.ascontiguousarray(wkT).astype(NPDT),
                "wvd": np.ascontiguousarray(wvT).astype(NPDT),
                "wcd": np.ascontiguousarray(wcT).astype(NPDT),
                "cosd": cosT,
                "sind": sinT,
                "rotd": _rot_perm().astype(NPDT),
                "trid": tri.astype(NPDT),
                "identd": np.eye(P, dtype=np.float32).astype(NPDT),
            }
        )
    return in_maps


_NC = None


def get_nc():
    global _NC
    if _NC is None:
        _NC = build_bass()
    return _NC


def run_cores(in_maps, core_ids, **kw):
    return run_bass_kernel_spmd(get_nc(), in_maps, core_ids=core_ids, **kw)


def kernel(x, Wq, Wk, Wv, Wc, position_ids, _trace=False, _res_out=None):
    in_maps = make_in_maps(x, Wq, Wk, Wv, Wc, position_ids)
    res = run_cores(in_maps, list(range(8)), trace=_trace)
    if _res_out is not None:
        _res_out.append(res)
    outs = [res.results[i]["out"].astype(np.float32) for i in range(8)]
    y = np.stack(
        [
            outs[0] + outs[1] + outs[2] + outs[3],
            outs[4] + outs[5] + outs[6] + outs[7],
        ]
    )
    return y
